# revision 8
# baseline (speedup 1.0000x reference)
"""Bilinear grid-sample kernel for Trainium2 (Bass/Tile), batch-parallel over 8 NeuronCores.

im:   [8, 512, 512, 16] f32 NHWC
grid: [8, 2, 512, 512]  f32, coords in [-1, 1] (x = grid[:,0], y = grid[:,1])
out:  [8, 512, 512, 16] f32

Each core handles one batch image:
  1. Build a full-patch scratch in DRAM: entry(y, x) = 64 floats
     [im[y,x], im[y,x+1], im[y+1,x], im[y+1,x+1]] via shifted on-chip copies.
     (Entries at x=511 / y=511 hold garbage in the shifted slots; never read
     because x0 <= 510 and y0 <= 510 after clipping.)
  2. Compute x0/y0/wx1/wy1 and idx = y0*512 + x0 on DVE.
  3. Gather one 256B patch per output pixel with [P,1]-offset
     indirect_dma_start (128 pixels per instruction; the HW DGE uses the
     dest row size == 64 elements as the index stride, matching the scratch
     entry size).
  4. Bilinear blend on DVE with per-(partition, column) weights broadcast
     over the 16 channels, then store contiguous output runs.
"""

import sys

import numpy as np

sys.path.insert(0, "/opt/trn_rl_repo")

from concourse import bacc, bass, mybir, tile
from concourse.bass import IndirectOffsetOnAxis
from concourse.bass_utils import run_bass_kernel_spmd

F32 = mybir.dt.float32
I32 = mybir.dt.int32
ALU = mybir.AluOpType

H = W = 512
C = 16
P = 128
NPP = (H * W) // P  # 2048 pixel-columns per partition-row
GB = 128  # gather columns per blend batch
NB = NPP // GB  # 16 blend batches
MAGIC = 8388608.0  # 2^23: (x + MAGIC) - MAGIC rounds fp32 to nearest integer


def _build_scratch(nc, sc_d, im_d, tc):
    """scratch[y*512+x] = [im[y,x], im[y,x+1], im[y+1,x], im[y+1,x+1]] (64 f32)."""
    with tc.tile_pool(name="bld", bufs=1) as bp:
        # batches of 127 output rows from 128 loaded rows
        starts = [0, 127, 254, 381]
        for r in starts:
            a = bp.tile([127, W * C], F32, tag="a")
            nc.sync.dma_start(
                out=a[:], in_=im_d[r : r + 127, :, :].rearrange("r x c -> r (x c)")
            )
            a1 = bp.tile([127, W * C], F32, tag="a1")
            nc.sync.dma_start(
                out=a1[:], in_=im_d[r + 1 : r + 128, :, :].rearrange("r x c -> r (x c)")
            )
            for h in range(2):
                s = bp.tile([127, 256 * 64], F32, tag="s")
                sv = s[:].rearrange("p (x e) -> p x e", e=64)
                xo = 256 * h * C
                # corner (y, x)
                nc.vector.tensor_copy(
                    out=sv[:, :, 0:16],
                    in_=a[0:127, xo : xo + 4096].rearrange("p (x c) -> p x c", c=16),
                )
                # corner (y, x+1); at x=511 the source would be off the end -> skip last col
                nx = 256 if h == 0 else 255
                if nx == 255:
                    nc.vector.memset(sv[:, 255:256, 16:32], 0.0)
                    nc.vector.memset(sv[:, 255:256, 48:64], 0.0)
                nc.vector.tensor_copy(
                    out=sv[:, 0:nx, 16:32],
                    in_=a[0:127, xo + 16 : xo + 16 + nx * 16].rearrange(
                        "p (x c) -> p x c", c=16
                    ),
                )
                # corner (y+1, x)
                nc.vector.tensor_copy(
                    out=sv[:, :, 32:48],
                    in_=a1[0:127, xo : xo + 4096].rearrange("p (x c) -> p x c", c=16),
                )
                # corner (y+1, x+1)
                nc.vector.tensor_copy(
                    out=sv[:, 0:nx, 48:64],
                    in_=a1[0:127, xo + 16 : xo + 16 + nx * 16].rearrange(
                        "p (x c) -> p x c", c=16
                    ),
                )
                nc.sync.dma_start(
                    out=sc_d[r : r + 127, h * 256 : (h + 1) * 256, :].rearrange(
                        "y x e -> y (x e)"
                    ),
                    in_=s[:],
                )
        # tail rows 508..510 (3 entry rows, uses im rows 508..511)
        a = bp.tile([127, W * C], F32, tag="a")
        nc.sync.dma_start(
            out=a[0:3, :], in_=im_d[508:511, :, :].rearrange("r x c -> r (x c)")
        )
        a1 = bp.tile([127, W * C], F32, tag="a1")
        nc.sync.dma_start(
            out=a1[0:3, :], in_=im_d[509:512, :, :].rearrange("r x c -> r (x c)")
        )
        for h in range(2):
            s = bp.tile([127, 256 * 64], F32, tag="s")
            sv = s[:].rearrange("p (x e) -> p x e", e=64)
            xo = 256 * h * C
            nx = 256 if h == 0 else 255
            if nx == 255:
                nc.vector.memset(sv[0:3, 255:256, 16:32], 0.0)
                nc.vector.memset(sv[0:3, 255:256, 48:64], 0.0)
            nc.vector.tensor_copy(
                out=sv[0:3, :, 0:16],
                in_=a[0:3, xo : xo + 4096].rearrange("p (x c) -> p x c", c=16),
            )
            nc.vector.tensor_copy(
                out=sv[0:3, 0:nx, 16:32],
                in_=a[0:3, xo + 16 : xo + 16 + nx * 16].rearrange(
                    "p (x c) -> p x c", c=16
                ),
            )
            nc.vector.tensor_copy(
                out=sv[0:3, :, 32:48],
                in_=a1[0:3, xo : xo + 4096].rearrange("p (x c) -> p x c", c=16),
            )
            nc.vector.tensor_copy(
                out=sv[0:3, 0:nx, 48:64],
                in_=a1[0:3, xo + 16 : xo + 16 + nx * 16].rearrange(
                    "p (x c) -> p x c", c=16
                ),
            )
            nc.sync.dma_start(
                out=sc_d[508:511, h * 256 : (h + 1) * 256, :].rearrange(
                    "y x e -> y (x e)"
                ),
                in_=s[0:3, :],
            )


def _build_program():
    nc = bacc.Bacc(
        "TRN2", target_bir_lowering=False, debug=False, enable_asserts=False
    )

    im_d = nc.dram_tensor("im", [H, W, C], F32, kind="ExternalInput")
    gx_d = nc.dram_tensor("gx", [P, NPP], F32, kind="ExternalInput")
    gy_d = nc.dram_tensor("gy", [P, NPP], F32, kind="ExternalInput")
    out_d = nc.dram_tensor("out", [P, NPP * C], F32, kind="ExternalOutput")
    sc_d = nc.dram_tensor("scratch", [H, W, 64], F32)

    with tile.TileContext(nc) as tc:
        _build_scratch(nc, sc_d, im_d, tc)

        with tc.tile_pool(name="persist", bufs=1) as pp:
            wx1 = pp.tile([P, NPP], F32, tag="wx1")
            wy1 = pp.tile([P, NPP], F32, tag="wy1")
            idx_i = pp.tile([P, NPP], I32, tag="idx")

            with tc.tile_pool(name="scratchp", bufs=1) as sp:

                def axis_setup(src_dram, x0_tag, w1_out):
                    raw = sp.tile([P, NPP], F32, tag="s1")
                    nc.sync.dma_start(out=raw[:], in_=src_dram[:])
                    g = sp.tile([P, NPP], F32, tag="s2")
                    nc.vector.tensor_scalar(
                        out=g[:], in0=raw[:], scalar1=1.0, scalar2=256.0,
                        op0=ALU.add, op1=ALU.mult,
                    )
                    t = sp.tile([P, NPP], F32, tag="s3")
                    nc.vector.tensor_scalar(
                        out=t[:], in0=g[:], scalar1=0.0, scalar2=510.5,
                        op0=ALU.max, op1=ALU.min,
                    )
                    r = sp.tile([P, NPP], F32, tag="s1")
                    nc.vector.tensor_scalar(
                        out=r[:], in0=t[:], scalar1=MAGIC, scalar2=MAGIC,
                        op0=ALU.add, op1=ALU.subtract,
                    )
                    d = sp.tile([P, NPP], F32, tag="s4")
                    nc.vector.tensor_tensor(out=d[:], in0=r[:], in1=t[:], op=ALU.is_gt)
                    x0 = sp.tile([P, NPP], F32, tag=x0_tag)
                    nc.vector.tensor_tensor(
                        out=x0[:], in0=r[:], in1=d[:], op=ALU.subtract
                    )
                    nc.vector.tensor_tensor(
                        out=w1_out[:], in0=g[:], in1=x0[:], op=ALU.subtract
                    )
                    return x0

                x0f = axis_setup(gx_d, "x0x", wx1)
                y0f = axis_setup(gy_d, "x0y", wy1)

                idxf = sp.tile([P, NPP], F32, tag="s1")
                nc.vector.scalar_tensor_tensor(
                    out=idxf[:], in0=y0f[:], scalar=float(W), in1=x0f[:],
                    op0=ALU.mult, op1=ALU.add,
                )
                nc.vector.tensor_copy(out=idx_i[:], in_=idxf[:])

            with (
                tc.tile_pool(name="gather", bufs=2) as gp,
                tc.tile_pool(name="work", bufs=2) as wp,
                tc.tile_pool(name="wts", bufs=2) as wtp,
            ):
                for b in range(NB):
                    tb = gp.tile([P, GB, 64], F32, tag="tb")
                    for gi in range(GB):
                        n = b * GB + gi
                        nc.gpsimd.indirect_dma_start(
                            out=tb[:, gi, :],
                            out_offset=None,
                            in_=sc_d[:],
                            in_offset=IndirectOffsetOnAxis(
                                ap=idx_i[:, n : n + 1], axis=1
                            ),
                            element_offset=0,
                        )

                    sl = slice(b * GB, (b + 1) * GB)
                    m = wtp.tile([P, GB, 1], F32, tag="m")
                    nc.vector.tensor_tensor(
                        out=m[:, :, 0], in0=wx1[:, sl], in1=wy1[:, sl], op=ALU.mult
                    )
                    w10 = wtp.tile([P, GB, 1], F32, tag="w10")
                    nc.vector.tensor_tensor(
                        out=w10[:, :, 0], in0=wx1[:, sl], in1=m[:, :, 0],
                        op=ALU.subtract,
                    )
                    w01 = wtp.tile([P, GB, 1], F32, tag="w01")
                    nc.vector.tensor_tensor(
                        out=w01[:, :, 0], in0=wy1[:, sl], in1=m[:, :, 0],
                        op=ALU.subtract,
                    )
                    u = wtp.tile([P, GB, 1], F32, tag="u")
                    nc.vector.tensor_tensor(
                        out=u[:, :, 0], in0=m[:, :, 0], in1=wx1[:, sl], op=ALU.subtract
                    )
                    w00 = wtp.tile([P, GB, 1], F32, tag="w00")
                    nc.vector.scalar_tensor_tensor(
                        out=w00[:, :, 0], in0=u[:, :, 0], scalar=1.0, in1=wy1[:, sl],
                        op0=ALU.add, op1=ALU.subtract,
                    )

                    shp = [P, GB, C]
                    a = wp.tile(shp, F32, tag="a")
                    bb = wp.tile(shp, F32, tag="b")
                    nc.vector.tensor_tensor(
                        out=a[:], in0=tb[:, :, 0:16], in1=w00[:].to_broadcast(shp),
                        op=ALU.mult,
                    )
                    nc.vector.tensor_tensor(
                        out=bb[:], in0=tb[:, :, 16:32], in1=w10[:].to_broadcast(shp),
                        op=ALU.mult,
                    )
                    nc.vector.tensor_tensor(out=a[:], in0=a[:], in1=bb[:], op=ALU.add)
                    nc.vector.tensor_tensor(
                        out=bb[:], in0=tb[:, :, 32:48], in1=w01[:].to_broadcast(shp),
                        op=ALU.mult,
                    )
                    nc.vector.tensor_tensor(out=a[:], in0=a[:], in1=bb[:], op=ALU.add)
                    nc.vector.tensor_tensor(
                        out=bb[:], in0=tb[:, :, 48:64], in1=m[:].to_broadcast(shp),
                        op=ALU.mult,
                    )
                    nc.vector.tensor_tensor(out=a[:], in0=a[:], in1=bb[:], op=ALU.add)

                    nc.sync.dma_start(
                        out=out_d[:, b * GB * C : (b + 1) * GB * C],
                        in_=a[:, :, :],
                    )

    nc.compile()
    return nc


_NC = None


def _get_nc():
    global _NC
    if _NC is None:
        _NC = _build_program()
    return _NC


def _run(im, grid, trace=False):
    nc = _get_nc()
    bsz = im.shape[0]
    in_maps = []
    for b in range(bsz):
        in_maps.append(
            {
                "im": np.ascontiguousarray(im[b]),
                "gx": np.ascontiguousarray(grid[b, 0]).reshape(P, NPP),
                "gy": np.ascontiguousarray(grid[b, 1]).reshape(P, NPP),
            }
        )
    res = run_bass_kernel_spmd(nc, in_maps, list(range(bsz)), trace=trace)
    out = np.stack(
        [res.results[b]["out"].reshape(H, W, C) for b in range(bsz)], axis=0
    )
    return out, res


def kernel(im, grid):
    out, _ = _run(np.asarray(im), np.asarray(grid))
    return out
